# revision 22
# baseline (speedup 1.0000x reference)
"""Trainium2 Bass kernel for nn_MoE_32332513804634.

MoE: 16 routed experts (top-6, softmax-then-bias routing) + dense shared
expert, T=4096 tokens, D=2048, H=1408, HS=2816, fp32.

Strategy (8 NeuronCores, SPMD):
  - Host computes the gate (cheap) and per-expert token lists.
  - Expert parallelism as a per-core list of variable-width token chunks
    (width compiled in, identical multiset on every core; each chunk binds
    one expert's weights via its own dram tensors). Experts are cut into
    near-equal pieces and pieces are sorted+grouped 8-at-a-time into slots
    so the compiled capacity is within ~3% of the 3072/core lower bound
    (vs ~33% padding waste for fixed 2048/512 slot caps).
  - Each chunk runs SwiGLU with bf16 matmuls accumulating in fp32 PSUM,
    the per-token combine weight applied as a per-partition DVE scale on
    the PSUM->SBUF copy.  Outputs are written in bf16 (halves output DMA).
  - Shared expert is tensor-parallel over its 2816 hidden dim (352 rows
    per core, padded to 384); its weights are SBUF-resident (loaded once,
    reused by all 8 token chunks).  Shared chunks are interleaved between
    routed chunks to smooth DMA pressure.
  - Host scatters chunk outputs back to token rows, sums partials, and
    adds the second-layer biases (cw*b2 per expert, bs2 once) in fp32.
"""

import sys
import numpy as np

sys.path.insert(0, "/opt/trn_rl_repo")

import concourse.bass as bass  # noqa: E402
import concourse.tile as tile  # noqa: E402
from concourse import bacc, mybir  # noqa: E402
from concourse.bass_utils import run_bass_kernel_spmd  # noqa: E402

T = 4096
D = 2048
H = 1408
E = 16
TOP_K = 6
HS = 2816
N_CORES = 8
HM = H // 128          # 11
KO = D // 128          # 16
HS_PAD = 384           # shared hidden shard (352) padded to 3*128
HMS = HS_PAD // 128    # 3
F32 = mybir.dt.float32
BF16 = mybir.dt.bfloat16
MM_DT = BF16

_PROGRAM_CACHE: dict = {}


def _to_mm(a):
    import ml_dtypes
    return np.ascontiguousarray(a).astype(ml_dtypes.bfloat16)


def _host_gate(xf, gate_w, gate_b):
    scores = xf @ gate_w.T
    m = scores.max(axis=-1, keepdims=True)
    p = np.exp(scores - m, dtype=np.float32)
    probs = p / p.sum(axis=-1, keepdims=True)
    biased = probs + gate_b
    idx = np.argpartition(biased, E - TOP_K, axis=1)[:, E - TOP_K:]
    mask = np.zeros((xf.shape[0], E), dtype=bool)
    mask[np.arange(xf.shape[0])[:, None], idx] = True
    cw = np.where(mask, probs, 0.0).astype(np.float32)
    toks = [np.flatnonzero(mask[:, e]).astype(np.int64) for e in range(E)]
    return cw, toks


def _chunk_cost(w):
    """Approx PE cost (ns) of one compiled chunk of width w."""
    l1 = 11 * 16 * 2 * max(107.0, w / 2.4 + 16)
    l2 = 4 * ((w + 127) // 128) * 11 * (512 / 2.4 + 16)
    return l1 + l2


def _cut_pieces(counts, target):
    """Cut each expert into near-equal pieces (each <= 512)."""
    pieces = []
    for e, c in enumerate(counts):
        c = int(c)
        if c == 0:
            continue
        k = max(1, -(-c // target))
        while -(-c // k) > 512:
            k += 1
        base, rem = divmod(c, k)
        start = 0
        for i in range(k):
            n = base + (1 if i < rem else 0)
            pieces.append((n, e, start))
            start += n
    return pieces


def _cut_pieces_base(counts, base_sz):
    """Cut into pieces of base_sz plus one ragged final piece per expert."""
    pieces = []
    for e, c in enumerate(counts):
        c = int(c)
        start = 0
        while c >= base_sz + 128:
            pieces.append((base_sz, e, start))
            start += base_sz
            c -= base_sz
        if c > 512:
            h1 = (c + 1) // 2
            pieces.append((h1, e, start))
            start += h1
            c -= h1
        if c > 0:
            pieces.append((c, e, start))
    return pieces


def _plan_groupsort(counts):
    """Equal-cut pieces, sorted and grouped 8-at-a-time into slots."""
    best = None
    cand = [_cut_pieces(counts, t) for t in range(320, 513, 8)]
    cand += [_cut_pieces_base(counts, b) for b in (512, 448, 384)]
    for pieces in cand:
        ps = sorted(pieces, key=lambda p: -p[0])
        nslots = -(-len(ps) // N_CORES)
        widths = []
        for s in range(nslots):
            grp = ps[s * N_CORES:(s + 1) * N_CORES]
            w = -(-max(p[0] for p in grp) // 16) * 16
            widths.append(w)
        cost = sum(_chunk_cost(w) for w in widths)
        if best is None or cost < best[0]:
            best = (cost, tuple(widths), ps)
    cost, widths, ps = best
    assignment = [[None] * len(widths) for _ in range(N_CORES)]
    for i, (n, e, st) in enumerate(ps):
        s, c = divmod(i, N_CORES)
        assignment[c][s] = (e, st, n)
    return cost, widths, assignment


def _solve_bundles3(nz, Ws, Is):
    """Exact DP: pick one (i, j, k) bundle per expert with per-width slot
    budgets Is. Returns list of (waste, (i, j, k)) per expert or None."""
    W1, W2, W3 = Ws
    I1, I2, I3 = Is
    opts = []
    for e, c in nz:
        o = []
        for i in range(0, min(I1, -(-c // W1)) + 1):
            r1 = c - i * W1
            jmax = min(I2, max(0, -(-r1 // W2))) if W2 else 0
            for j in range(0, jmax + 1):
                r2 = r1 - j * W2
                k = max(0, -(-r2 // W3)) if W3 else 0
                if k > I3 or (not W3 and r2 > 0):
                    continue
                o.append((i * W1 + j * W2 + k * W3 - c, (i, j, k)))
        if not o:
            return None
        opts.append(sorted(set(o)))
    reach = [np.zeros((I1 + 1, I2 + 1, I3 + 1), dtype=bool)]
    reach[0][0, 0, 0] = True
    for o in opts:
        cur = reach[-1]
        nxt = np.zeros_like(cur)
        for _, (i, j, k) in o:
            nxt[i:, j:, k:] |= cur[:I1 + 1 - i, :I2 + 1 - j, :I3 + 1 - k]
        if not nxt.any():
            return None
        reach.append(nxt)
    si, sj, sk = np.argwhere(reach[-1])[0]
    pick = [None] * len(opts)
    for idx in range(len(opts) - 1, -1, -1):
        for w, (i, j, k) in opts[idx]:
            if i <= si and j <= sj and k <= sk and \
                    reach[idx][si - i, sj - j, sk - k]:
                pick[idx] = (w, (i, j, k))
                si, sj, sk = si - i, sj - j, sk - k
                break
        if pick[idx] is None:
            return None
    return pick


def _plan_twowidth(counts):
    """Per-core multiset of up to 3 chunk widths; experts assigned slot
    bundles via exact DP; configs tried in ascending PE-cost order."""
    nz = [(e, int(c)) for e, c in enumerate(counts) if c > 0]
    total = sum(c for _, c in nz)
    sizes = (512, 448, 384, 320, 256, 192, 128)
    configs = []
    seen = set()
    from itertools import combinations
    for ws in list(combinations(sizes, 2)) + list(combinations(sizes, 3)):
        W1, W2 = ws[0], ws[1]
        W3 = ws[2] if len(ws) == 3 else 0
        for a in range(0, 9):
            for b in range(0, 11):
                for cc in range(0, 9 if W3 else 1):
                    cap = a * W1 + b * W2 + cc * W3
                    if cap * N_CORES < total or cap > 4608:
                        continue
                    key = tuple(sorted([W1] * a + [W2] * b + [W3] * cc))
                    if key in seen:
                        continue
                    seen.add(key)
                    cost = (a * _chunk_cost(W1) + b * _chunk_cost(W2) +
                            (cc * _chunk_cost(W3) if W3 else 0))
                    configs.append((cost, W1, W2, W3, a, b, cc))
    configs.sort()
    best = None
    for cost, W1, W2, W3, a, b, cc in configs:
        pick = _solve_bundles3(nz, (W1, W2, W3),
                               (a * N_CORES, b * N_CORES, cc * N_CORES))
        if pick is not None:
            best = (cost, (W1, W2, W3, a, b, cc), pick)
            break
    if best is None:
        return None
    cost, (W1, W2, W3, a, b, cc), pick = best
    widths = (W1,) * a + (W2,) * b + ((W3,) * cc if W3 else ())
    slots = [[(core, s) for s in range(a) for core in range(N_CORES)],
             [(core, s + a) for s in range(b) for core in range(N_CORES)],
             [(core, s + a + b) for s in range(cc) for core in range(N_CORES)]]
    ptr = [0, 0, 0]
    Wv = (W1, W2, W3)
    assignment = [[None] * len(widths) for _ in range(N_CORES)]
    for (e, c), (w, ijk) in zip(nz, pick):
        start = 0
        rem = c
        for lvl in range(3):
            for _ in range(ijk[lvl]):
                core, s = slots[lvl][ptr[lvl]]; ptr[lvl] += 1
                n = min(rem, Wv[lvl])
                if n > 0:
                    assignment[core][s] = (e, start, n)
                start += n
                rem -= n
        assert rem == 0, (e, c, rem)
    return cost, widths, assignment


def _plan(counts):
    """Returns (widths, assignment): widths = per-core compiled chunk
    widths; assignment[core][slot] = (expert, start, fill) or None."""
    plans = [_plan_groupsort(counts)]
    tw = _plan_twowidth(counts)
    if tw is not None:
        plans.append(tw)
    plans.sort(key=lambda p: p[0])
    _, widths, assignment = plans[0]
    return tuple(widths), assignment


def _build_program(widths):
    nc = bacc.Bacc("TRN2", debug=False, num_devices=N_CORES)

    ins = {}
    outs = {}

    def din(name, shape, dt=MM_DT):
        ins[name] = nc.dram_tensor(name, list(shape), dt, kind="ExternalInput").ap()
        return ins[name]

    def dout(name, shape, dt=BF16):
        outs[name] = nc.dram_tensor(name, list(shape), dt, kind="ExternalOutput").ap()
        return outs[name]

    for s, w in enumerate(widths):
        ntch = -(-w // 128)
        din(f"xg{s}", (D, w))
        din(f"w1t{s}", (D, H))
        din(f"w3t{s}", (D, H))
        din(f"w2ta{s}", (H, D))
        din(f"b1_{s}", (128, HM), F32)
        din(f"b3_{s}", (128, HM), F32)
        din(f"scl{s}", (128, ntch), F32)
        dout(f"oe{s}", (ntch * 128, D))
    # shared expert: 2 chunks/core, each 512 tokens x 1408 hidden
    # (2816 = 2 x H, so shared chunks are shape-identical to routed ones)
    for u in range(2):
        din(f"xs{u}", (D, 512))
        din(f"ws1h{u}", (D, H))
        din(f"ws3h{u}", (D, H))
        din(f"ws2h{u}", (H, D))
        din(f"bsh1_{u}", (128, HM), F32)
        din(f"bsh3_{u}", (128, HM), F32)
        dout(f"zs{u}", (512, D))

    with tile.TileContext(nc) as tc:
        with (
            tc.tile_pool(name="xpool", bufs=2) as xpool,
            tc.tile_pool(name="hpool", bufs=2) as hpool,
            tc.tile_pool(name="wcol", bufs=3) as wcol,
            tc.tile_pool(name="w2pool", bufs=2) as w2pool,
            tc.tile_pool(name="tmp", bufs=2) as tmp,
            tc.tile_pool(name="opool", bufs=4) as opool,
            tc.tile_pool(name="cpool", bufs=1) as cpool,
            tc.tile_pool(name="pp", bufs=2, space="PSUM") as pp,
        ):
            def mlp_chunk(xg_ap, w1_ap, w3_ap, w2_ap, b1_ap, b3_ap, scl_ap,
                          out_ap, w, n_hm, wtag):
                """One chunk: out[:w] = scale * (swiglu(xg) @ W2^T)."""
                ntch = -(-w // 128)
                x3 = xg_ap.rearrange("(ko p) t -> p ko t", p=128)
                w1c3 = w1_ap.rearrange("(ko p) h -> p ko h", p=128)
                w3c3 = w3_ap.rearrange("(ko p) h -> p ko h", p=128)
                w23 = w2_ap.rearrange("(k p) d -> p k d", p=128)

                b1sb = cpool.tile([128, n_hm], F32, tag=f"b1{wtag}")
                nc.sync.dma_start(b1sb[:], b1_ap)
                b3sb = cpool.tile([128, n_hm], F32, tag=f"b3{wtag}")
                nc.sync.dma_start(b3sb[:], b3_ap)
                if scl_ap is not None:
                    sclsb = cpool.tile([128, ntch], F32, tag=f"scl{wtag}")
                    nc.sync.dma_start(sclsb[:], scl_ap)

                xsb = xpool.tile([128, KO, 512], MM_DT, tag="xg")
                nc.sync.dma_start(xsb[:, :, :w], x3)
                nw2 = 4 if n_hm == HM else 2
                w2sbs = []
                hsb = hpool.tile([128, n_hm, 512], MM_DT, tag=f"h{n_hm}",
                                 bufs=(2 if n_hm == HM else 1))
                for hm in range(n_hm):
                    if hm == 2:
                        # prefetch W2 tiles once L1-critical DMAs are queued;
                        # they stream during the rest of L1
                        for dm in range(4):
                            w2sb = w2pool.tile([128, n_hm, 512], MM_DT,
                                               tag=f"w2s{n_hm}", bufs=nw2)
                            nc.sync.dma_start(
                                w2sb[:], w23[:, :, dm * 512:(dm + 1) * 512])
                            w2sbs.append(w2sb)
                    w1t_ = wcol.tile([128, KO, 128], MM_DT, tag="w1c")
                    nc.sync.dma_start(w1t_[:], w1c3[:, :, hm * 128:(hm + 1) * 128])
                    w3t_ = wcol.tile([128, KO, 128], MM_DT, tag="w3c")
                    nc.sync.dma_start(w3t_[:], w3c3[:, :, hm * 128:(hm + 1) * 128])
                    ps1 = pp.tile([128, 512], F32, tag="ph1")
                    for ko in range(KO):
                        nc.tensor.matmul(ps1[:, :w], w1t_[:, ko, :], xsb[:, ko, :w],
                                         start=(ko == 0), stop=(ko == KO - 1))
                    ps3 = pp.tile([128, 512], F32, tag="ph3")
                    for ko in range(KO):
                        nc.tensor.matmul(ps3[:, :w], w3t_[:, ko, :], xsb[:, ko, :w],
                                         start=(ko == 0), stop=(ko == KO - 1))
                    h1t = tmp.tile([128, 512], F32, tag="h1t")
                    nc.scalar.activation(h1t[:, :w], ps1[:, :w],
                                         mybir.ActivationFunctionType.Silu,
                                         bias=b1sb[:, hm:hm + 1])
                    h3t = tmp.tile([128, 512], F32, tag="h3t")
                    nc.scalar.activation(h3t[:, :w], ps3[:, :w],
                                         mybir.ActivationFunctionType.Identity,
                                         bias=b3sb[:, hm:hm + 1])
                    nc.vector.tensor_mul(hsb[:, hm, :w], h1t[:, :w], h3t[:, :w])
                # second matmul: out rows = tokens
                for dm in range(4):
                    w2sb = w2sbs[dm]
                    for tch in range(ntch):
                        tok0 = tch * 128
                        tcw = min(128, w - tok0)
                        ps2 = pp.tile([128, 512], F32, tag="po", bufs=4)
                        for k in range(n_hm):
                            lhsT = hsb[:, k, tok0:tok0 + tcw]
                            nc.tensor.matmul(ps2[:tcw, :], lhsT, w2sb[:, k, :],
                                             start=(k == 0), stop=(k == n_hm - 1))
                        osb = opool.tile([128, 512], BF16, tag="osb")
                        if scl_ap is not None:
                            nc.vector.tensor_scalar_mul(
                                osb[:tcw, :], ps2[:tcw, :], sclsb[:tcw, tch:tch + 1])
                        else:
                            nc.vector.tensor_copy(osb[:tcw, :], ps2[:tcw, :])
                        nc.sync.dma_start(
                            out_ap[tok0:tok0 + tcw, dm * 512:(dm + 1) * 512],
                            osb[:tcw, :])

            def routed_chunk(s, w):
                mlp_chunk(ins[f"xg{s}"], ins[f"w1t{s}"], ins[f"w3t{s}"],
                          ins[f"w2ta{s}"], ins[f"b1_{s}"], ins[f"b3_{s}"],
                          ins[f"scl{s}"], outs[f"oe{s}"], w, HM, f"e{s}")

            def shared_chunk(u):
                mlp_chunk(ins[f"xs{u}"], ins[f"ws1h{u}"], ins[f"ws3h{u}"],
                          ins[f"ws2h{u}"], ins[f"bsh1_{u}"], ins[f"bsh3_{u}"],
                          None, outs[f"zs{u}"], 512, HM, f"sh{u}")

            # shared chunks at 1/3 and 2/3 of the routed sequence
            n_r = len(widths)
            for i in range(n_r):
                routed_chunk(i, widths[i])
                if i == n_r // 3:
                    shared_chunk(0)
                if i == (2 * n_r) // 3:
                    shared_chunk(1)

    nc.compile()
    return nc


def kernel(x, gate_w, gate_b, w1, b1, w2, b2, w3, b3,
           ws1, bs1, ws2, bs2, ws3, bs3):
    x = np.asarray(x, np.float32)
    xf = np.ascontiguousarray(x.reshape(-1, D))
    gate_w = np.asarray(gate_w, np.float32)
    gate_b = np.asarray(gate_b, np.float32)
    w1 = np.asarray(w1, np.float32)
    b1 = np.asarray(b1, np.float32)
    w2 = np.asarray(w2, np.float32)
    b2 = np.asarray(b2, np.float32)
    w3 = np.asarray(w3, np.float32)
    b3 = np.asarray(b3, np.float32)
    ws1 = np.asarray(ws1, np.float32)
    bs1 = np.asarray(bs1, np.float32)
    ws2 = np.asarray(ws2, np.float32)
    bs2 = np.asarray(bs2, np.float32)
    ws3 = np.asarray(ws3, np.float32)
    bs3 = np.asarray(bs3, np.float32)

    cw, toks = _host_gate(xf, gate_w, gate_b)
    counts = np.array([len(t) for t in toks])
    widths, assignment = _plan(counts)

    if widths not in _PROGRAM_CACHE:
        _PROGRAM_CACHE[widths] = _build_program(widths)
    nc = _PROGRAM_CACHE[widths]

    xT = np.ascontiguousarray(xf.T)  # [D, T]
    xT_mm = _to_mm(xT)

    w1t = {}
    w3t = {}
    w2ta = {}
    b1t = {}
    b3t = {}
    need = sorted({p[0] for slots in assignment for p in slots if p is not None})
    for e in need:
        w1t[e] = _to_mm(w1[e].T)
        w3t[e] = _to_mm(w3[e].T)
        w2ta[e] = _to_mm(w2[e].T)
        b1t[e] = np.ascontiguousarray(b1[e].reshape(HM, 128).T)
        b3t[e] = np.ascontiguousarray(b3[e].reshape(HM, 128).T)

    ws1h = [_to_mm(ws1[u * H:(u + 1) * H].T) for u in range(2)]
    ws3h = [_to_mm(ws3[u * H:(u + 1) * H].T) for u in range(2)]
    ws2h = [_to_mm(ws2[:, u * H:(u + 1) * H].T) for u in range(2)]
    bs1h = [np.ascontiguousarray(bs1[u * H:(u + 1) * H].reshape(HM, 128).T)
            for u in range(2)]
    bs3h = [np.ascontiguousarray(bs3[u * H:(u + 1) * H].reshape(HM, 128).T)
            for u in range(2)]

    in_maps = []
    for c in range(N_CORES):
        m = {}
        for s, w in enumerate(widths):
            ntch = -(-w // 128)
            piece = assignment[c][s]
            xg = np.zeros((D, w), np.float32)
            scl = np.zeros(ntch * 128, np.float32)
            if piece is None:
                e = need[0]
            else:
                e, s0, n = piece
                tk = toks[e][s0:s0 + n]
                xg[:, :n] = xT[:, tk]
                scl[:n] = cw[tk, e]
            m[f"w1t{s}"] = w1t[e]
            m[f"w3t{s}"] = w3t[e]
            m[f"w2ta{s}"] = w2ta[e]
            m[f"b1_{s}"] = b1t[e]
            m[f"b3_{s}"] = b3t[e]
            m[f"xg{s}"] = _to_mm(xg)
            m[f"scl{s}"] = np.ascontiguousarray(scl.reshape(ntch, 128).T)
        # shared expert: chunk u=0 -> token block c, hidden half 0;
        # chunk u=1 -> token block (c+4)%8, hidden half 1
        for u in range(2):
            tb = c if u == 0 else (c + 4) % N_CORES
            m[f"xs{u}"] = np.ascontiguousarray(
                xT_mm[:, tb * 512:(tb + 1) * 512])
            m[f"ws1h{u}"] = ws1h[u]
            m[f"ws3h{u}"] = ws3h[u]
            m[f"ws2h{u}"] = ws2h[u]
            m[f"bsh1_{u}"] = bs1h[u]
            m[f"bsh3_{u}"] = bs3h[u]
        in_maps.append(m)

    res = run_bass_kernel_spmd(nc, in_maps, list(range(N_CORES)))

    y = np.zeros((T, D), np.float32)
    for c in range(N_CORES):
        for s, w in enumerate(widths):
            piece = assignment[c][s]
            if piece is None:
                continue
            e, s0, n = piece
            tk = toks[e][s0:s0 + n]
            y[tk] += res.results[c][f"oe{s}"][:n].astype(np.float32)
            y[tk] += cw[tk, e][:, None] * b2[e][None, :]
        for u in range(2):
            tb = c if u == 0 else (c + 4) % N_CORES
            y[tb * 512:(tb + 1) * 512] += \
                res.results[c][f"zs{u}"].astype(np.float32)
    y += bs2[None, :]
    return y.reshape(x.shape).astype(np.float32)


# revision 29
# speedup vs baseline: 1.0614x; 1.0614x over previous
"""Trainium2 Bass kernel for nn_MoE_32332513804634.

MoE: 16 routed experts (top-6, softmax-then-bias routing) + dense shared
expert, T=4096 tokens, D=2048, H=1408, HS=2816, fp32.

Strategy (8 NeuronCores, SPMD):
  - Host computes the gate (cheap) and per-expert token lists.
  - Expert parallelism as a per-core list of variable-width token chunks
    (width compiled in, identical multiset on every core; each chunk binds
    one expert's weights via its own dram tensors). Experts are cut into
    near-equal pieces and pieces are sorted+grouped 8-at-a-time into slots
    so the compiled capacity is within ~3% of the 3072/core lower bound
    (vs ~33% padding waste for fixed 2048/512 slot caps).
  - Each chunk runs SwiGLU with bf16 matmuls accumulating in fp32 PSUM,
    the per-token combine weight applied as a per-partition DVE scale on
    the PSUM->SBUF copy.  Outputs are written in bf16 (halves output DMA).
  - Shared expert is tensor-parallel over its 2816 hidden dim (352 rows
    per core, padded to 384); its weights are SBUF-resident (loaded once,
    reused by all 8 token chunks).  Shared chunks are interleaved between
    routed chunks to smooth DMA pressure.
  - Host scatters chunk outputs back to token rows, sums partials, and
    adds the second-layer biases (cw*b2 per expert, bs2 once) in fp32.
"""

import sys
import numpy as np

sys.path.insert(0, "/opt/trn_rl_repo")

import concourse.bass as bass  # noqa: E402
import concourse.tile as tile  # noqa: E402
from concourse import bacc, mybir  # noqa: E402
from concourse.bass_utils import run_bass_kernel_spmd  # noqa: E402

T = 4096
D = 2048
H = 1408
E = 16
TOP_K = 6
HS = 2816
N_CORES = 8
HM = H // 128          # 11
KO = D // 128          # 16
HS_PAD = 384           # shared hidden shard (352) padded to 3*128
HMS = HS_PAD // 128    # 3
F32 = mybir.dt.float32
BF16 = mybir.dt.bfloat16
MM_DT = BF16

_PROGRAM_CACHE: dict = {}


def _to_mm(a):
    import ml_dtypes
    return np.ascontiguousarray(a).astype(ml_dtypes.bfloat16)


def _host_gate(xf, gate_w, gate_b):
    scores = xf @ gate_w.T
    m = scores.max(axis=-1, keepdims=True)
    p = np.exp(scores - m, dtype=np.float32)
    probs = p / p.sum(axis=-1, keepdims=True)
    biased = probs + gate_b
    idx = np.argpartition(biased, E - TOP_K, axis=1)[:, E - TOP_K:]
    mask = np.zeros((xf.shape[0], E), dtype=bool)
    mask[np.arange(xf.shape[0])[:, None], idx] = True
    cw = np.where(mask, probs, 0.0).astype(np.float32)
    toks = [np.flatnonzero(mask[:, e]).astype(np.int64) for e in range(E)]
    return cw, toks


def _chunk_cost(w):
    """Approx PE cost (ns) of one compiled chunk of width w."""
    l1 = 11 * 16 * 2 * max(107.0, w / 2.4 + 16)
    l2 = 4 * ((w + 127) // 128) * 11 * (512 / 2.4 + 16)
    return l1 + l2


def _cut_pieces(counts, target):
    """Cut each expert into near-equal pieces (each <= 512)."""
    pieces = []
    for e, c in enumerate(counts):
        c = int(c)
        if c == 0:
            continue
        k = max(1, -(-c // target))
        while -(-c // k) > 512:
            k += 1
        base, rem = divmod(c, k)
        start = 0
        for i in range(k):
            n = base + (1 if i < rem else 0)
            pieces.append((n, e, start))
            start += n
    return pieces


def _cut_pieces_base(counts, base_sz):
    """Cut into pieces of base_sz plus one ragged final piece per expert."""
    pieces = []
    for e, c in enumerate(counts):
        c = int(c)
        start = 0
        while c >= base_sz + 128:
            pieces.append((base_sz, e, start))
            start += base_sz
            c -= base_sz
        if c > 512:
            h1 = (c + 1) // 2
            pieces.append((h1, e, start))
            start += h1
            c -= h1
        if c > 0:
            pieces.append((c, e, start))
    return pieces


def _plan_groupsort(counts):
    """Equal-cut pieces, sorted and grouped 8-at-a-time into slots."""
    best = None
    cand = [_cut_pieces(counts, t) for t in range(320, 513, 8)]
    cand += [_cut_pieces_base(counts, b) for b in (512, 448, 384)]
    for pieces in cand:
        ps = sorted(pieces, key=lambda p: -p[0])
        nslots = -(-len(ps) // N_CORES)
        widths = []
        for s in range(nslots):
            grp = ps[s * N_CORES:(s + 1) * N_CORES]
            w = -(-max(p[0] for p in grp) // 16) * 16
            widths.append(w)
        cost = sum(_chunk_cost(w) for w in widths)
        if best is None or cost < best[0]:
            best = (cost, tuple(widths), ps)
    cost, widths, ps = best
    assignment = [[None] * len(widths) for _ in range(N_CORES)]
    for i, (n, e, st) in enumerate(ps):
        s, c = divmod(i, N_CORES)
        assignment[c][s] = (e, st, n)
    return cost, widths, assignment


def _solve_bundles3(nz, Ws, Is):
    """Exact DP: pick one (i, j, k) bundle per expert with per-width slot
    budgets Is. Returns list of (waste, (i, j, k)) per expert or None."""
    W1, W2, W3 = Ws
    I1, I2, I3 = Is
    opts = []
    for e, c in nz:
        o = []
        for i in range(0, min(I1, -(-c // W1)) + 1):
            r1 = c - i * W1
            jmax = min(I2, max(0, -(-r1 // W2))) if W2 else 0
            for j in range(0, jmax + 1):
                r2 = r1 - j * W2
                k = max(0, -(-r2 // W3)) if W3 else 0
                if k > I3 or (not W3 and r2 > 0):
                    continue
                o.append((i * W1 + j * W2 + k * W3 - c, (i, j, k)))
        if not o:
            return None
        opts.append(sorted(set(o)))
    reach = [np.zeros((I1 + 1, I2 + 1, I3 + 1), dtype=bool)]
    reach[0][0, 0, 0] = True
    for o in opts:
        cur = reach[-1]
        nxt = np.zeros_like(cur)
        for _, (i, j, k) in o:
            nxt[i:, j:, k:] |= cur[:I1 + 1 - i, :I2 + 1 - j, :I3 + 1 - k]
        if not nxt.any():
            return None
        reach.append(nxt)
    si, sj, sk = np.argwhere(reach[-1])[0]
    pick = [None] * len(opts)
    for idx in range(len(opts) - 1, -1, -1):
        for w, (i, j, k) in opts[idx]:
            if i <= si and j <= sj and k <= sk and \
                    reach[idx][si - i, sj - j, sk - k]:
                pick[idx] = (w, (i, j, k))
                si, sj, sk = si - i, sj - j, sk - k
                break
        if pick[idx] is None:
            return None
    return pick


def _plan_twowidth(counts):
    """Per-core multiset of up to 3 chunk widths; experts assigned slot
    bundles via exact DP; configs tried in ascending PE-cost order."""
    nz = [(e, int(c)) for e, c in enumerate(counts) if c > 0]
    total = sum(c for _, c in nz)
    sizes = (512, 448, 384, 320, 256, 192, 128)
    configs = []
    seen = set()
    from itertools import combinations
    for ws in list(combinations(sizes, 2)) + list(combinations(sizes, 3)):
        W1, W2 = ws[0], ws[1]
        W3 = ws[2] if len(ws) == 3 else 0
        for a in range(0, 9):
            for b in range(0, 11):
                for cc in range(0, 9 if W3 else 1):
                    cap = a * W1 + b * W2 + cc * W3
                    if cap * N_CORES < total or cap > 4608:
                        continue
                    key = tuple(sorted([W1] * a + [W2] * b + [W3] * cc))
                    if key in seen:
                        continue
                    seen.add(key)
                    cost = (a * _chunk_cost(W1) + b * _chunk_cost(W2) +
                            (cc * _chunk_cost(W3) if W3 else 0))
                    configs.append((cost, W1, W2, W3, a, b, cc))
    configs.sort()
    best = None
    for cost, W1, W2, W3, a, b, cc in configs:
        pick = _solve_bundles3(nz, (W1, W2, W3),
                               (a * N_CORES, b * N_CORES, cc * N_CORES))
        if pick is not None:
            best = (cost, (W1, W2, W3, a, b, cc), pick)
            break
    if best is None:
        return None
    cost, (W1, W2, W3, a, b, cc), pick = best
    widths = (W1,) * a + (W2,) * b + ((W3,) * cc if W3 else ())
    slots = [[(core, s) for s in range(a) for core in range(N_CORES)],
             [(core, s + a) for s in range(b) for core in range(N_CORES)],
             [(core, s + a + b) for s in range(cc) for core in range(N_CORES)]]
    ptr = [0, 0, 0]
    Wv = (W1, W2, W3)
    assignment = [[None] * len(widths) for _ in range(N_CORES)]
    for (e, c), (w, ijk) in zip(nz, pick):
        start = 0
        rem = c
        for lvl in range(3):
            for _ in range(ijk[lvl]):
                core, s = slots[lvl][ptr[lvl]]; ptr[lvl] += 1
                n = min(rem, Wv[lvl])
                if n > 0:
                    assignment[core][s] = (e, start, n)
                start += n
                rem -= n
        assert rem == 0, (e, c, rem)
    return cost, widths, assignment


def _plan(counts):
    """Returns (widths, assignment): widths = per-core compiled chunk
    widths; assignment[core][slot] = (expert, start, fill) or None."""
    plans = [_plan_groupsort(counts)]
    tw = _plan_twowidth(counts)
    if tw is not None:
        plans.append(tw)
    plans.sort(key=lambda p: p[0])
    _, widths, assignment = plans[0]
    return tuple(widths), assignment


def _build_program(widths):
    nc = bacc.Bacc("TRN2", debug=False, num_devices=N_CORES)

    ins = {}
    outs = {}

    def din(name, shape, dt=MM_DT):
        ins[name] = nc.dram_tensor(name, list(shape), dt, kind="ExternalInput").ap()
        return ins[name]

    def dout(name, shape, dt=BF16):
        outs[name] = nc.dram_tensor(name, list(shape), dt, kind="ExternalOutput").ap()
        return outs[name]

    for s, w in enumerate(widths):
        ntch = -(-w // 128)
        din(f"xg{s}", (D, w))
        din(f"w1t{s}", (D, H))
        din(f"w3t{s}", (D, H))
        din(f"w2ta{s}", (H, D))
        din(f"b1_{s}", (128, HM), F32)
        din(f"b3_{s}", (128, HM), F32)
        din(f"scl{s}", (128, ntch), F32)
        dout(f"oe{s}", (ntch * 128, D))
    # shared expert: hidden-sharded (352 -> pad 384 per core), weights
    # SBUF-resident, 8 light token chunks interleaved as DMA relief
    din("xt", (D, T))
    din("ws1s", (D, HS_PAD))
    din("ws3s", (D, HS_PAD))
    din("ws2sa", (HS_PAD, D))
    din("bs1", (128, HMS), F32)
    din("bs3", (128, HMS), F32)
    dout("zs", (T, D))

    with tile.TileContext(nc) as tc:
        with (
            tc.tile_pool(name="xpool", bufs=2) as xpool,
            tc.tile_pool(name="hpool", bufs=2) as hpool,
            tc.tile_pool(name="wcol", bufs=3) as wcol,
            tc.tile_pool(name="w2pool", bufs=2) as w2pool,
            tc.tile_pool(name="tmp", bufs=2) as tmp,
            tc.tile_pool(name="opool", bufs=4) as opool,
            tc.tile_pool(name="cpool", bufs=1) as cpool,
            tc.tile_pool(name="pp", bufs=2, space="PSUM") as pp,
        ):
            def mlp_chunk(xg_ap, w1_ap, w3_ap, w2_ap, b1_ap, b3_ap, scl_ap,
                          out_ap, w, n_hm, wtag):
                """One chunk: out[:w] = scale * (swiglu(xg) @ W2^T)."""
                ntch = -(-w // 128)
                x3 = xg_ap.rearrange("(ko p) t -> p ko t", p=128)
                w1c3 = w1_ap.rearrange("(ko p) h -> p ko h", p=128)
                w3c3 = w3_ap.rearrange("(ko p) h -> p ko h", p=128)
                w23 = w2_ap.rearrange("(k p) d -> p k d", p=128)

                b1sb = cpool.tile([128, n_hm], F32, tag=f"b1{wtag}")
                nc.sync.dma_start(b1sb[:], b1_ap)
                b3sb = cpool.tile([128, n_hm], F32, tag=f"b3{wtag}")
                nc.sync.dma_start(b3sb[:], b3_ap)
                if scl_ap is not None:
                    sclsb = cpool.tile([128, ntch], F32, tag=f"scl{wtag}")
                    nc.sync.dma_start(sclsb[:], scl_ap)

                xsb = xpool.tile([128, KO, 512], MM_DT, tag="xg")
                nc.sync.dma_start(xsb[:, :, :w], x3)
                w2sbs = []
                hsb = hpool.tile([128, n_hm, 512], MM_DT, tag=f"h{n_hm}",
                                 bufs=2)
                for hm in range(n_hm):
                    if hm in (2, 4, 6, 8):
                        # prefetch one W2 tile; spread across the L1 loop so
                        # the bursts don't starve the w1c/w3c stream
                        dm = (hm - 2) // 2
                        w2sb = w2pool.tile([128, n_hm, 512], MM_DT,
                                           tag=f"w2s{n_hm}", bufs=5)
                        nc.sync.dma_start(
                            w2sb[:], w23[:, :, dm * 512:(dm + 1) * 512])
                        w2sbs.append(w2sb)
                    w1t_ = wcol.tile([128, KO, 128], MM_DT, tag="w1c", bufs=5)
                    nc.sync.dma_start(w1t_[:], w1c3[:, :, hm * 128:(hm + 1) * 128])
                    w3t_ = wcol.tile([128, KO, 128], MM_DT, tag="w3c", bufs=5)
                    nc.sync.dma_start(w3t_[:], w3c3[:, :, hm * 128:(hm + 1) * 128])
                    ps1 = pp.tile([128, 512], F32, tag="ph1")
                    for ko in range(KO):
                        nc.tensor.matmul(ps1[:, :w], w1t_[:, ko, :], xsb[:, ko, :w],
                                         start=(ko == 0), stop=(ko == KO - 1))
                    ps3 = pp.tile([128, 512], F32, tag="ph3")
                    for ko in range(KO):
                        nc.tensor.matmul(ps3[:, :w], w3t_[:, ko, :], xsb[:, ko, :w],
                                         start=(ko == 0), stop=(ko == KO - 1))
                    h1t = tmp.tile([128, 512], F32, tag="h1t")
                    nc.scalar.activation(h1t[:, :w], ps1[:, :w],
                                         mybir.ActivationFunctionType.Silu,
                                         bias=b1sb[:, hm:hm + 1])
                    h3t = tmp.tile([128, 512], F32, tag="h3t")
                    nc.scalar.activation(h3t[:, :w], ps3[:, :w],
                                         mybir.ActivationFunctionType.Identity,
                                         bias=b3sb[:, hm:hm + 1])
                    nc.vector.tensor_mul(hsb[:, hm, :w], h1t[:, :w], h3t[:, :w])
                # second matmul: out rows = tokens
                for dm in range(4):
                    w2sb = w2sbs[dm]
                    for tch in range(ntch):
                        tok0 = tch * 128
                        tcw = min(128, w - tok0)
                        ps2 = pp.tile([128, 512], F32, tag="po", bufs=4)
                        for k in range(n_hm):
                            lhsT = hsb[:, k, tok0:tok0 + tcw]
                            nc.tensor.matmul(ps2[:tcw, :], lhsT, w2sb[:, k, :],
                                             start=(k == 0), stop=(k == n_hm - 1))
                        osb = opool.tile([128, 512], BF16, tag="osb")
                        if scl_ap is not None:
                            nc.vector.tensor_scalar_mul(
                                osb[:tcw, :], ps2[:tcw, :], sclsb[:tcw, tch:tch + 1])
                        else:
                            nc.vector.tensor_copy(osb[:tcw, :], ps2[:tcw, :])
                        nc.sync.dma_start(
                            out_ap[tok0:tok0 + tcw, dm * 512:(dm + 1) * 512],
                            osb[:tcw, :])

            # shared-expert residents (emitted piecewise between early chunks)
            ws1r = cpool.tile([128, KO, HS_PAD], MM_DT, tag="ws1r")
            ws3r = cpool.tile([128, KO, HS_PAD], MM_DT, tag="ws3r")
            ws2r = cpool.tile([128, HMS, D], MM_DT, tag="ws2r")
            bs1r = cpool.tile([128, HMS], F32, tag="bs1r")
            bs3r = cpool.tile([128, HMS], F32, tag="bs3r")
            resident_loads = [
                lambda: nc.sync.dma_start(
                    ws1r[:], ins["ws1s"].rearrange("(ko p) h -> p ko h", p=128)),
                lambda: nc.sync.dma_start(
                    ws3r[:], ins["ws3s"].rearrange("(ko p) h -> p ko h", p=128)),
                lambda: (nc.sync.dma_start(
                    ws2r[:], ins["ws2sa"].rearrange("(k p) d -> p k d", p=128)),
                    nc.sync.dma_start(bs1r[:], ins["bs1"]),
                    nc.sync.dma_start(bs3r[:], ins["bs3"])),
            ]

            def routed_chunk(s, w):
                mlp_chunk(ins[f"xg{s}"], ins[f"w1t{s}"], ins[f"w3t{s}"],
                          ins[f"w2ta{s}"], ins[f"b1_{s}"], ins[f"b3_{s}"],
                          ins[f"scl{s}"], outs[f"oe{s}"], w, HM, f"e{s}")

            def shared_chunk(t):
                xt3 = ins["xt"].rearrange("(ko p) t -> p ko t", p=128)
                xsb = xpool.tile([128, KO, 512], MM_DT, tag="xg")
                nc.sync.dma_start(xsb[:], xt3[:, :, t * 512:(t + 1) * 512])
                hsb = hpool.tile([128, HM, 512], MM_DT, tag=f"h{HM}", bufs=2)
                for hm in range(HMS):
                    ps1 = pp.tile([128, 512], F32, tag="ph1")
                    for ko in range(KO):
                        nc.tensor.matmul(ps1[:], ws1r[:, ko, hm * 128:(hm + 1) * 128],
                                         xsb[:, ko, :],
                                         start=(ko == 0), stop=(ko == KO - 1))
                    ps3 = pp.tile([128, 512], F32, tag="ph3")
                    for ko in range(KO):
                        nc.tensor.matmul(ps3[:], ws3r[:, ko, hm * 128:(hm + 1) * 128],
                                         xsb[:, ko, :],
                                         start=(ko == 0), stop=(ko == KO - 1))
                    h1t = tmp.tile([128, 512], F32, tag="h1t")
                    nc.scalar.activation(h1t[:], ps1[:],
                                         mybir.ActivationFunctionType.Silu,
                                         bias=bs1r[:, hm:hm + 1])
                    h3t = tmp.tile([128, 512], F32, tag="h3t")
                    nc.scalar.activation(h3t[:], ps3[:],
                                         mybir.ActivationFunctionType.Identity,
                                         bias=bs3r[:, hm:hm + 1])
                    nc.vector.tensor_mul(hsb[:, hm, :], h1t[:], h3t[:])
                for dm in range(4):
                    for tch in range(4):
                        tok0 = t * 512 + tch * 128
                        ps2 = pp.tile([128, 512], F32, tag="po", bufs=4)
                        for k in range(HMS):
                            lhsT = hsb[:, k, tch * 128:(tch + 1) * 128]
                            nc.tensor.matmul(ps2[:], lhsT,
                                             ws2r[:, k, dm * 512:(dm + 1) * 512],
                                             start=(k == 0), stop=(k == HMS - 1))
                        osb = opool.tile([128, 512], BF16, tag="osb")
                        nc.vector.tensor_copy(osb[:], ps2[:])
                        nc.sync.dma_start(
                            outs["zs"][tok0:tok0 + 128, dm * 512:(dm + 1) * 512],
                            osb[:])

            # interleave routed chunks with shared chunks to smooth DMA
            n_r, n_s = len(widths), T // 512
            ri, si = 0, 0
            seq = []
            while ri < n_r or si < n_s:
                if ri < n_r:
                    seq.append(("r", ri)); ri += 1
                while si < n_s and (ri >= n_r or (si + 1) * n_r <= (ri + 1) * n_s):
                    seq.append(("s", si)); si += 1
            for n_emitted, (kind, i) in enumerate(seq):
                if kind == "r":
                    routed_chunk(i, widths[i])
                else:
                    shared_chunk(i)
                if n_emitted == 0:
                    for ld in resident_loads:
                        ld()

    nc.compile()
    return nc


def kernel(x, gate_w, gate_b, w1, b1, w2, b2, w3, b3,
           ws1, bs1, ws2, bs2, ws3, bs3):
    x = np.asarray(x, np.float32)
    xf = np.ascontiguousarray(x.reshape(-1, D))
    gate_w = np.asarray(gate_w, np.float32)
    gate_b = np.asarray(gate_b, np.float32)
    w1 = np.asarray(w1, np.float32)
    b1 = np.asarray(b1, np.float32)
    w2 = np.asarray(w2, np.float32)
    b2 = np.asarray(b2, np.float32)
    w3 = np.asarray(w3, np.float32)
    b3 = np.asarray(b3, np.float32)
    ws1 = np.asarray(ws1, np.float32)
    bs1 = np.asarray(bs1, np.float32)
    ws2 = np.asarray(ws2, np.float32)
    bs2 = np.asarray(bs2, np.float32)
    ws3 = np.asarray(ws3, np.float32)
    bs3 = np.asarray(bs3, np.float32)

    cw, toks = _host_gate(xf, gate_w, gate_b)
    counts = np.array([len(t) for t in toks])
    widths, assignment = _plan(counts)

    if widths not in _PROGRAM_CACHE:
        _PROGRAM_CACHE[widths] = _build_program(widths)
    nc = _PROGRAM_CACHE[widths]

    xT = np.ascontiguousarray(xf.T)  # [D, T]
    xT_mm = _to_mm(xT)

    w1t = {}
    w3t = {}
    w2ta = {}
    b1t = {}
    b3t = {}
    need = sorted({p[0] for slots in assignment for p in slots if p is not None})
    for e in need:
        w1t[e] = _to_mm(w1[e].T)
        w3t[e] = _to_mm(w3[e].T)
        w2ta[e] = _to_mm(w2[e].T)
        b1t[e] = np.ascontiguousarray(b1[e].reshape(HM, 128).T)
        b3t[e] = np.ascontiguousarray(b3[e].reshape(HM, 128).T)

    hs_per = HS // N_CORES  # 352

    in_maps = []
    for c in range(N_CORES):
        m = {}
        for s, w in enumerate(widths):
            ntch = -(-w // 128)
            piece = assignment[c][s]
            xg = np.zeros((D, w), np.float32)
            scl = np.zeros(ntch * 128, np.float32)
            if piece is None:
                e = need[0]
            else:
                e, s0, n = piece
                tk = toks[e][s0:s0 + n]
                xg[:, :n] = xT[:, tk]
                scl[:n] = cw[tk, e]
            m[f"w1t{s}"] = w1t[e]
            m[f"w3t{s}"] = w3t[e]
            m[f"w2ta{s}"] = w2ta[e]
            m[f"b1_{s}"] = b1t[e]
            m[f"b3_{s}"] = b3t[e]
            m[f"xg{s}"] = _to_mm(xg)
            m[f"scl{s}"] = np.ascontiguousarray(scl.reshape(ntch, 128).T)
        # shared expert shard (352 hidden rows, padded to 384)
        r0 = c * hs_per
        ws1p = np.zeros((D, HS_PAD), np.float32)
        ws1p[:, :hs_per] = ws1[r0:r0 + hs_per].T
        ws3p = np.zeros((D, HS_PAD), np.float32)
        ws3p[:, :hs_per] = ws3[r0:r0 + hs_per].T
        ws2a = np.zeros((HS_PAD, D), np.float32)
        ws2a[:hs_per] = ws2[:, r0:r0 + hs_per].T
        bs1p = np.zeros(HS_PAD, np.float32)
        bs1p[:hs_per] = bs1[r0:r0 + hs_per]
        bs3p = np.zeros(HS_PAD, np.float32)
        bs3p[:hs_per] = bs3[r0:r0 + hs_per]
        m["xt"] = xT_mm
        m["ws1s"] = _to_mm(ws1p)
        m["ws3s"] = _to_mm(ws3p)
        m["ws2sa"] = _to_mm(ws2a)
        m["bs1"] = np.ascontiguousarray(bs1p.reshape(HMS, 128).T)
        m["bs3"] = np.ascontiguousarray(bs3p.reshape(HMS, 128).T)
        in_maps.append(m)

    res = run_bass_kernel_spmd(nc, in_maps, list(range(N_CORES)))

    y = np.zeros((T, D), np.float32)
    for c in range(N_CORES):
        for s, w in enumerate(widths):
            piece = assignment[c][s]
            if piece is None:
                continue
            e, s0, n = piece
            tk = toks[e][s0:s0 + n]
            y[tk] += res.results[c][f"oe{s}"][:n].astype(np.float32)
            y[tk] += cw[tk, e][:, None] * b2[e][None, :]
        y += res.results[c]["zs"].astype(np.float32)
    y += bs2[None, :]
    return y.reshape(x.shape).astype(np.float32)
